# revision 28
# baseline (speedup 1.0000x reference)
"""Locally banded sparse attention (window=64) on 8 Trainium2 NeuronCores.

Sequence-parallel: each core owns 256 contiguous query positions and
receives a 384-row x chunk (its 256 rows + 64-row halo on each side,
zero-padded at the sequence edges) plus a full replica of the four
projection matrices.  No device collectives are needed.

All matmuls run in bf16 (fp32 PSUM accumulation).  Attention scores are
computed directly in transposed layout S^T[key, query] = kT.T @ qT, and
the P@V matmul uses P^T as the stationary operand so its output lands
query-major: av[q, d] with the softmax denominator Z[q] riding along as a
ones-column of V (col 64 of each head's 65-wide slot).  Normalization is
then a per-partition reciprocal + tensor_scalar multiply — no partition
broadcasts anywhere.  The q-major attention output is PE-transposed back
to d-major for the output projection.

Host-side folds: SCALE and bq into Wq/bq, bv into an effective bo
(out += bv @ Wo.T is query-independent).

Engine balance: PE matmuls; ACT exp + kT/o copies; DVE qT/vaug copies,
reciprocals, normalization, transpose copies; GPSIMD band-mask multiplies;
inputs DMA'd on two HWDGE queues (sync + scalar) in consumption order.
"""

import numpy as np
import ml_dtypes

import concourse.bass as bass
import concourse.tile as tile
from concourse import bacc, mybir
from concourse import bass_utils

F32 = mybir.dt.float32
BF16 = mybir.dt.bfloat16
N_CORES = 8
S = 2048
D = 512
H = 8
DK = 64
W = 64
SCALE = 1.0 / np.sqrt(DK)
SEQ_PER_CORE = S // N_CORES          # 256
CHUNK = SEQ_PER_CORE + 2 * W         # 384 rows of k/v context per core

_CACHE = {}


def _build_program():
    nc = bacc.Bacc("TRN2", target_bir_lowering=False, debug=False,
                   num_devices=N_CORES)

    # packed DRAM inputs (bf16): weight chunk kk lives at cols [kk*512, +512)
    x4 = nc.dram_tensor("x4", [128, 4 * CHUNK], BF16, kind="ExternalInput").ap()
    wk4 = nc.dram_tensor("wk4", [128, 2048], BF16, kind="ExternalInput").ap()
    wq4 = nc.dram_tensor("wq4", [128, 2048], BF16, kind="ExternalInput").ap()
    wv4 = nc.dram_tensor("wv4", [128, 2048], BF16, kind="ExternalInput").ap()
    wo4 = nc.dram_tensor("wo4", [128, 2048], BF16, kind="ExternalInput").ap()
    # binary band masks, S^T layout: block (t, kb) at cols [(t*2+kb)*128)
    mask4 = nc.dram_tensor("mask4", [128, 512], BF16, kind="ExternalInput").ap()
    # col 0-3: bk[g]; col 4-7: bo_eff[g]; col 8-11: bq_scaled[g]
    bias = nc.dram_tensor("bias", [128, 12], F32, kind="ExternalInput").ap()
    identw = nc.dram_tensor("identw", [128, 128], BF16, kind="ExternalInput").ap()
    outT = nc.dram_tensor("outT", [128, 4 * SEQ_PER_CORE], BF16,
                          kind="ExternalOutput").ap()

    with tile.TileContext(nc) as tc:
        with (
            tc.tile_pool(name="const", bufs=1) as cpool,
            tc.tile_pool(name="pp", bufs=2, space="PSUM") as pp,
            tc.tile_pool(name="s_ps", bufs=4, space="PSUM") as s_ps,
            tc.tile_pool(name="av_ps", bufs=2, space="PSUM") as av_ps,
            tc.tile_pool(name="soft", bufs=4) as soft,
            tc.tile_pool(name="small", bufs=4) as small,
        ):
            def persist(shape, tag, dtype=BF16):
                return cpool.tile(shape, dtype, tag=tag, name=tag)

            x_sb = persist([128, 4 * CHUNK], "x")
            wk_sb = persist([128, 2048], "wk")
            wq_sb = persist([128, 2048], "wq")
            wv_sb = persist([128, 2048], "wv")
            wo_sb = persist([128, 2048], "wo")
            mask_sb = persist([128, 512], "mask")
            bias_sb = persist([128, 12], "bias", F32)
            k_sb = [persist([128, CHUNK], f"k{g}") for g in range(4)]
            q_sb = [persist([128, SEQ_PER_CORE], f"q{g}") for g in range(4)]
            # v with a ones column per head: head h at cols [h*65, +64], Z at h*65+64
            vaug = [persist([128, 8 * 65], f"v{r}") for r in range(3)]
            aT_sb = [persist([128, D], f"aT{t}") for t in range(2)]
            a_sb = [persist([128, SEQ_PER_CORE], f"a{g}") for g in range(4)]
            o_all = persist([128, 4 * SEQ_PER_CORE], "o_all")
            ident = persist([128, 128], "ident")
            scratch = persist([128, 256], "scratch")

            # input DMAs on two HWDGE queues: x and the first wk group
            # chunk race first in parallel (SDMA round-robins across
            # queues), everything else queues behind them
            nc.sync.dma_start(x_sb[:], x4[:, :])
            nc.sync.dma_start(wq_sb[:], wq4[:, :])
            nc.sync.dma_start(wo_sb[:], wo4[:, :])
            nc.sync.dma_start(ident[:], identw[:, :])
            for g in range(4):   # wk4 packed g-major: group g at cols g*512
                nc.scalar.dma_start(wk_sb[:, g * 512:(g + 1) * 512],
                                    wk4[:, g * 512:(g + 1) * 512])
            nc.scalar.dma_start(bias_sb[:], bias[:, :])
            nc.scalar.dma_start(wv_sb[:], wv4[:, :])
            nc.scalar.dma_start(mask_sb[:], mask4[:, :])

            def vaug_ap(r, col0, ncols):
                base = vaug[r][:]
                p_step = base.ap[0][0]
                return bass.AP(base.tensor, base.offset + col0,
                               [[p_step, 128], [65, 8], [1, ncols]])

            for r in range(3):
                nc.vector.memset(vaug_ap(r, 64, 1), 1.0)

            # HAM warm-up: keep the PE streaming dummy matmuls while the
            # weight DMAs land so real matmuls run at 2.4 GHz, not 1.2
            nc.vector.memset(scratch[:], 0.0)
            for w in range(44):
                wps = s_ps.tile([128, 256], F32, tag="s", name="warm")
                nc.tensor.matmul(wps[:], scratch[:, 0:128], scratch[:],
                                 start=True, stop=True)

            # ---- projections ----------------------------------------
            # kT[g]: [128 dout, 384 keys], bf16, + bk   (copy on ACT)
            for g in range(4):
                ps = pp.tile([128, 512], F32, tag="pp", name="pp")
                for kk in range(4):
                    nc.tensor.matmul(ps[:, :CHUNK],
                                     wk_sb[:, g * 512 + kk * 128:
                                           g * 512 + kk * 128 + 128],
                                     x_sb[:, kk * CHUNK:(kk + 1) * CHUNK],
                                     start=(kk == 0), stop=(kk == 3))
                nc.scalar.activation(k_sb[g][:], ps[:, :CHUNK],
                                     mybir.ActivationFunctionType.Identity,
                                     bias=bias_sb[:, g:g + 1])
            # qT[g]: [128 dout, 256 queries] (SCALE, bq folded) (copy on DVE)
            for g in range(4):
                ps = pp.tile([128, 512], F32, tag="pp", name="pp")
                for kk in range(4):
                    nc.tensor.matmul(ps[:, :SEQ_PER_CORE],
                                     wq_sb[:, kk * 512 + g * 128:
                                           kk * 512 + g * 128 + 128],
                                     x_sb[:, kk * CHUNK + W:
                                          kk * CHUNK + W + SEQ_PER_CORE],
                                     start=(kk == 0), stop=(kk == 3))
                nc.vector.tensor_scalar_add(q_sb[g][:], ps[:, :SEQ_PER_CORE],
                                            bias_sb[:, 8 + g:9 + g])
            # v natural [keys, dout] -> vaug 65-wide head slots (copy on DVE)
            for r in range(3):
                ps = pp.tile([128, 512], F32, tag="pp", name="pp")
                for kk in range(4):
                    nc.tensor.matmul(ps[:],
                                     x_sb[:, kk * CHUNK + r * 128:
                                          kk * CHUNK + r * 128 + 128],
                                     wv_sb[:, kk * 512:(kk + 1) * 512],
                                     start=(kk == 0), stop=(kk == 3))
                nc.vector.tensor_copy(vaug_ap(r, 0, 64), ps[:])

            # ---- banded attention (S^T scores, q-major AV) ----------
            # software pipeline: S^T for step i runs on PE while step i-1
            # finishes softmax on ACT/GPSIMD, then its AV matmuls issue.
            steps = [(g, t) for g in range(4) for t in range(2)]
            pend = None   # (g, t, pA, pB, avz)

            def emit_av(st):
                g, t, pA, pB, avz = st
                hA, hB = 2 * g, 2 * g + 1
                for kb in range(2):
                    nc.tensor.matmul(avz[:, 0:65],
                                     pA[:, kb * 128:(kb + 1) * 128],
                                     vaug[t + kb][:, hA * 65:hA * 65 + 65],
                                     start=(kb == 0), stop=(kb == 1))
                for kb in range(2):
                    nc.tensor.matmul(avz[:, 65:130],
                                     pB[:, kb * 128:(kb + 1) * 128],
                                     vaug[t + kb][:, hB * 65:hB * 65 + 65],
                                     start=(kb == 0), stop=(kb == 1))
                rz2 = small.tile([128, 2], F32, tag="rz", name="rz")
                zbase = avz[:]
                pstep = zbase.ap[0][0]
                zin = bass.AP(zbase.tensor, zbase.offset + 64,
                              [[pstep, 128], [65, 2]])
                nc.vector.reciprocal(rz2[:], zin)
                # one multiply for both heads: avz cols {0:64, 65:129}
                # times rz2 broadcast 64-wide along the head axis
                avin = bass.AP(zbase.tensor, zbase.offset,
                               [[pstep, 128], [65, 2], [1, 64]])
                rzb = rz2[:]
                rzin = bass.AP(rzb.tensor, rzb.offset,
                               [[rzb.ap[0][0], 128], [1, 2], [0, 64]])
                nc.vector.tensor_mul(aT_sb[t][:, hA * 64:hA * 64 + 128],
                                     avin, rzin)

            for g, t in steps:
                sA = s_ps.tile([128, 256], F32, tag="s", name="s")
                sB = s_ps.tile([128, 256], F32, tag="s", name="s")
                for kb in range(2):
                    kc = (t + kb) * 128
                    nc.tensor.matmul(sA[:, kb * 128:(kb + 1) * 128],
                                     k_sb[g][0:64, kc:kc + 128],
                                     q_sb[g][0:64, t * 128:(t + 1) * 128],
                                     start=True, stop=True)
                    nc.tensor.matmul(sB[:, kb * 128:(kb + 1) * 128],
                                     k_sb[g][64:128, kc:kc + 128],
                                     q_sb[g][64:128, t * 128:(t + 1) * 128],
                                     start=True, stop=True)
                eA = soft.tile([128, 256], BF16, tag="eA", name="eA")
                eB = soft.tile([128, 256], BF16, tag="eB", name="eB")
                nc.scalar.activation(eA[:], sA[:],
                                     mybir.ActivationFunctionType.Exp)
                nc.scalar.activation(eB[:], sB[:],
                                     mybir.ActivationFunctionType.Exp)
                pA = soft.tile([128, 256], BF16, tag="pA", name="pA")
                pB = soft.tile([128, 256], BF16, tag="pB", name="pB")
                msk = mask_sb[:, t * 256:(t + 1) * 256]
                nc.gpsimd.tensor_mul(pA[:], eA[:], msk)
                nc.vector.tensor_mul(pB[:], eB[:], msk)
                avz = av_ps.tile([128, 130], F32, tag="av", name="av")
                if pend is not None:
                    emit_av(pend)
                pend = (g, t, pA, pB, avz)
            emit_av(pend)

            # ---- transpose a^T back to d-major ----------------------
            for t in range(2):
                for g in range(4):
                    tp = av_ps.tile([128, 128], BF16, tag="av", name="tp")
                    nc.tensor.transpose(tp[:],
                                        aT_sb[t][:, g * 128:(g + 1) * 128],
                                        ident[:])
                    nc.vector.tensor_copy(a_sb[g][:, t * 128:(t + 1) * 128],
                                          tp[:])

            # ---- output projection (tail, reuses pp banks) ----------
            for gg in range(4):
                ps = pp.tile([128, 512], F32, tag="pp", name="pp")
                for g in range(4):
                    nc.tensor.matmul(ps[:, :SEQ_PER_CORE],
                                     wo_sb[:, g * 512 + gg * 128:
                                           g * 512 + gg * 128 + 128],
                                     a_sb[g][:],
                                     start=(g == 0), stop=(g == 3))
                dst = o_all[:, gg * 256:(gg + 1) * 256]
                if gg % 2 == 0:
                    nc.scalar.activation(dst, ps[:, :SEQ_PER_CORE],
                                         mybir.ActivationFunctionType.Identity,
                                         bias=bias_sb[:, 4 + gg:5 + gg])
                else:
                    nc.vector.tensor_scalar_add(dst, ps[:, :SEQ_PER_CORE],
                                                bias_sb[:, 4 + gg:5 + gg])
            nc.sync.dma_start(outT[:, :], o_all[:])

    nc.compile()
    return nc


def _band_mask(T):
    """Binary S^T masks [128 keys, 256 (2 kb blocks of 128)] for global
    query tile T (0..15)."""
    j = np.arange(128)[:, None]
    r = np.arange(128)[None, :]
    out = np.zeros((128, 256), np.float32)
    for kb in range(2):
        band = (j >= r) if kb == 0 else (j <= r)
        jg = T * 128 - W + kb * 128 + j
        valid = band & (jg >= 0) & (jg < S)
        out[:, kb * 128:(kb + 1) * 128] = valid
    return out


def _prep_inputs(x, Wq, bq, Wk, bk, Wv, bv, Wo, bo):
    bf = ml_dtypes.bfloat16
    f32 = np.float32

    def pack_w(Wm, scale=1.0):
        wT = np.asarray(Wm, f32).T * scale          # [512 in, 512 out]
        return np.ascontiguousarray(
            wT.reshape(4, 128, 512).transpose(1, 0, 2).reshape(128, 2048)
            .astype(bf))

    # wk packed g-major (out-group g at cols g*512, k-chunk kk at +kk*128)
    # so per-group DMA chunks arrive in consumption order
    wkT = np.asarray(Wk, f32).T
    wk4 = np.ascontiguousarray(
        wkT.reshape(4, 128, 4, 128).transpose(1, 2, 0, 3).reshape(128, 2048)
        .astype(bf))
    wq4 = pack_w(Wq, SCALE)
    wv4 = pack_w(Wv)
    wo4 = pack_w(Wo)
    bo_eff = np.asarray(bo, f32) + np.asarray(Wo, f32) @ np.asarray(bv, f32)
    bias = np.zeros((128, 12), f32)
    bias[:, 0:4] = np.asarray(bk, f32).reshape(4, 128).T
    bias[:, 4:8] = bo_eff.reshape(4, 128).T
    bias[:, 8:12] = (np.asarray(bq, f32) * SCALE).reshape(4, 128).T

    identw = np.eye(128, dtype=f32).astype(bf)
    xf = np.asarray(x, f32)
    in_maps = []
    for c in range(N_CORES):
        klo = c * SEQ_PER_CORE - W
        lo, hi = max(0, klo), min(S, klo + CHUNK)
        xT_c = np.zeros((D, CHUNK), f32)
        xT_c[:, lo - klo:hi - klo] = xf[0, lo:hi, :].T
        x4 = np.ascontiguousarray(
            xT_c.reshape(4, 128, CHUNK).transpose(1, 0, 2)
            .reshape(128, 4 * CHUNK).astype(bf))
        m = np.concatenate([_band_mask(c * 2), _band_mask(c * 2 + 1)],
                           axis=1).astype(bf)
        in_maps.append({
            "x4": x4, "mask4": np.ascontiguousarray(m), "bias": bias,
            "wq4": wq4, "wk4": wk4, "wv4": wv4, "wo4": wo4,
            "identw": identw,
        })
    return in_maps


def kernel(x, Wq, bq, Wk, bk, Wv, bv, Wo, bo):
    if "nc" not in _CACHE:
        _CACHE["nc"] = _build_program()
    nc = _CACHE["nc"]
    in_maps = _prep_inputs(x, Wq, bq, Wk, bk, Wv, bv, Wo, bo)
    res = bass_utils.run_bass_kernel_spmd(nc, in_maps,
                                          core_ids=list(range(N_CORES)))
    out = np.empty((1, S, D), np.float32)
    for c in range(N_CORES):
        arr = np.asarray(res.results[c]["outT"]).astype(np.float32)
        chunk = arr.reshape(128, 4, SEQ_PER_CORE).transpose(1, 0, 2) \
                   .reshape(D, SEQ_PER_CORE).T
        out[0, c * SEQ_PER_CORE:(c + 1) * SEQ_PER_CORE, :] = chunk
    return out


# revision 31
# speedup vs baseline: 1.0467x; 1.0467x over previous
"""Locally banded sparse attention (window=64) on 8 Trainium2 NeuronCores.

Sequence-parallel: each core owns 256 contiguous query positions and
receives a 384-row x chunk (its 256 rows + 64-row halo on each side,
zero-padded at the sequence edges) plus a full replica of the four
projection matrices.  No device collectives are needed.

All matmuls run in bf16 (fp32 PSUM accumulation).  Attention scores are
computed directly in transposed layout S^T[key, query] = kT.T @ qT, and
the P@V matmul uses P^T as the stationary operand so its output lands
query-major: av[q, d] with the softmax denominator Z[q] riding along as a
ones-column of V (col 64 of each head's 65-wide slot).  Normalization is
then a per-partition reciprocal + tensor_scalar multiply — no partition
broadcasts anywhere.  The q-major attention output is PE-transposed back
to d-major for the output projection.

Host-side folds: SCALE and bq into Wq/bq, bv into an effective bo
(out += bv @ Wo.T is query-independent).

Engine balance: PE matmuls; ACT exp + kT/o copies; DVE qT/vaug copies,
reciprocals, normalization, transpose copies; GPSIMD band-mask multiplies;
inputs DMA'd on two HWDGE queues (sync + scalar) in consumption order.
"""

import numpy as np
import ml_dtypes

import concourse.bass as bass
import concourse.tile as tile
from concourse import bacc, mybir
from concourse import bass_utils

F32 = mybir.dt.float32
BF16 = mybir.dt.bfloat16
N_CORES = 8
S = 2048
D = 512
H = 8
DK = 64
W = 64
SCALE = 1.0 / np.sqrt(DK)
SEQ_PER_CORE = S // N_CORES          # 256
CHUNK = SEQ_PER_CORE + 2 * W         # 384 rows of k/v context per core

_CACHE = {}


def _build_program():
    nc = bacc.Bacc("TRN2", target_bir_lowering=False, debug=False,
                   num_devices=N_CORES)

    # packed DRAM inputs (bf16): weight chunk kk lives at cols [kk*512, +512)
    x4 = nc.dram_tensor("x4", [128, 4 * CHUNK], BF16, kind="ExternalInput").ap()
    wk4 = nc.dram_tensor("wk4", [128, 2048], BF16, kind="ExternalInput").ap()
    wq4 = nc.dram_tensor("wq4", [128, 2048], BF16, kind="ExternalInput").ap()
    wv4 = nc.dram_tensor("wv4", [128, 2048], BF16, kind="ExternalInput").ap()
    wo4 = nc.dram_tensor("wo4", [128, 2048], BF16, kind="ExternalInput").ap()
    # binary band masks, S^T layout: block (t, kb) at cols [(t*2+kb)*128)
    mask4 = nc.dram_tensor("mask4", [128, 512], BF16, kind="ExternalInput").ap()
    # col 0-3: bk[g]; col 4-7: bo_eff[g]; col 8-11: bq_scaled[g]
    bias = nc.dram_tensor("bias", [128, 12], F32, kind="ExternalInput").ap()
    identw = nc.dram_tensor("identw", [128, 128], BF16, kind="ExternalInput").ap()
    outT = nc.dram_tensor("outT", [128, 4 * SEQ_PER_CORE], BF16,
                          kind="ExternalOutput").ap()

    with tile.TileContext(nc) as tc:
        with (
            tc.tile_pool(name="const", bufs=1) as cpool,
            tc.tile_pool(name="pp", bufs=2, space="PSUM") as pp,
            tc.tile_pool(name="s_ps", bufs=4, space="PSUM") as s_ps,
            tc.tile_pool(name="av_ps", bufs=2, space="PSUM") as av_ps,
            tc.tile_pool(name="soft", bufs=4) as soft,
            tc.tile_pool(name="small", bufs=4) as small,
        ):
            def persist(shape, tag, dtype=BF16):
                return cpool.tile(shape, dtype, tag=tag, name=tag)

            x_sb = persist([128, 4 * CHUNK], "x")
            wk_sb = persist([128, 2048], "wk")
            wq_sb = persist([128, 2048], "wq")
            wv_sb = persist([128, 2048], "wv")
            wo_sb = persist([128, 2048], "wo")
            mask_sb = persist([128, 512], "mask")
            bias_sb = persist([128, 12], "bias", F32)
            k_sb = [persist([128, CHUNK], f"k{g}") for g in range(4)]
            q_sb = [persist([128, SEQ_PER_CORE], f"q{g}") for g in range(4)]
            # v with a ones column per head: head h at cols [h*65, +64], Z at h*65+64
            vaug = [persist([128, 8 * 65], f"v{r}") for r in range(3)]
            aT_sb = [persist([128, D], f"aT{t}") for t in range(2)]
            a_sb = [persist([128, SEQ_PER_CORE], f"a{g}") for g in range(4)]
            o_all = persist([128, 4 * SEQ_PER_CORE], "o_all")
            ident = persist([128, 128], "ident")
            scratch = persist([128, 256], "scratch")

            # input DMAs: single sync HWDGE ring, strict consumption order
            # (in-ring transfers complete FIFO; a second queue displaces
            # the framework const-loader onto sync and delays everything)
            nc.sync.dma_start(x_sb[:], x4[:, :])
            nc.sync.dma_start(wk_sb[:], wk4[:, :])
            nc.sync.dma_start(bias_sb[:], bias[:, :])
            nc.sync.dma_start(wq_sb[:], wq4[:, :])
            nc.sync.dma_start(wv_sb[:], wv4[:, :])
            nc.sync.dma_start(mask_sb[:], mask4[:, :])
            nc.sync.dma_start(ident[:], identw[:, :])
            nc.sync.dma_start(wo_sb[:], wo4[:, :])

            def vaug_ap(r, col0, ncols):
                base = vaug[r][:]
                p_step = base.ap[0][0]
                return bass.AP(base.tensor, base.offset + col0,
                               [[p_step, 128], [65, 8], [1, ncols]])

            for r in range(3):
                nc.vector.memset(vaug_ap(r, 64, 1), 1.0)

            # HAM warm-up: keep the PE streaming dummy matmuls while the
            # weight DMAs land so real matmuls run at 2.4 GHz, not 1.2
            nc.vector.memset(scratch[:], 0.0)
            for w in range(14):
                wps = s_ps.tile([128, 256], F32, tag="s", name="warm")
                nc.tensor.matmul(wps[:], scratch[:, 0:128], scratch[:],
                                 start=True, stop=True)

            # ---- projections ----------------------------------------
            # kT[g]: [128 dout, 384 keys], bf16, + bk   (copy on ACT)
            for g in range(4):
                ps = pp.tile([128, 512], F32, tag="pp", name="pp")
                for kk in range(4):
                    nc.tensor.matmul(ps[:, :CHUNK],
                                     wk_sb[:, g * 512 + kk * 128:
                                           g * 512 + kk * 128 + 128],
                                     x_sb[:, kk * CHUNK:(kk + 1) * CHUNK],
                                     start=(kk == 0), stop=(kk == 3))
                nc.scalar.activation(k_sb[g][:], ps[:, :CHUNK],
                                     mybir.ActivationFunctionType.Identity,
                                     bias=bias_sb[:, g:g + 1])
            # qT[g]: [128 dout, 256 queries] (SCALE, bq folded) (copy on DVE)
            for g in range(4):
                ps = pp.tile([128, 512], F32, tag="pp", name="pp")
                for kk in range(4):
                    nc.tensor.matmul(ps[:, :SEQ_PER_CORE],
                                     wq_sb[:, kk * 512 + g * 128:
                                           kk * 512 + g * 128 + 128],
                                     x_sb[:, kk * CHUNK + W:
                                          kk * CHUNK + W + SEQ_PER_CORE],
                                     start=(kk == 0), stop=(kk == 3))
                nc.vector.tensor_scalar_add(q_sb[g][:], ps[:, :SEQ_PER_CORE],
                                            bias_sb[:, 8 + g:9 + g])
            # v natural [keys, dout] -> vaug 65-wide head slots (copy on DVE)
            for r in range(3):
                ps = pp.tile([128, 512], F32, tag="pp", name="pp")
                for kk in range(4):
                    nc.tensor.matmul(ps[:],
                                     x_sb[:, kk * CHUNK + r * 128:
                                          kk * CHUNK + r * 128 + 128],
                                     wv_sb[:, kk * 512:(kk + 1) * 512],
                                     start=(kk == 0), stop=(kk == 3))
                nc.vector.tensor_copy(vaug_ap(r, 0, 64), ps[:])

            # ---- banded attention (S^T scores, q-major AV) ----------
            # software pipeline: S^T for step i runs on PE while step i-1
            # finishes softmax on ACT/GPSIMD, then its AV matmuls issue.
            steps = [(g, t) for g in range(4) for t in range(2)]
            pend = None   # (g, t, pA, pB, avz)

            def emit_av(st):
                g, t, pA, pB, avz = st
                hA, hB = 2 * g, 2 * g + 1
                for kb in range(2):
                    nc.tensor.matmul(avz[:, 0:65],
                                     pA[:, kb * 128:(kb + 1) * 128],
                                     vaug[t + kb][:, hA * 65:hA * 65 + 65],
                                     start=(kb == 0), stop=(kb == 1))
                for kb in range(2):
                    nc.tensor.matmul(avz[:, 65:130],
                                     pB[:, kb * 128:(kb + 1) * 128],
                                     vaug[t + kb][:, hB * 65:hB * 65 + 65],
                                     start=(kb == 0), stop=(kb == 1))
                rz2 = small.tile([128, 2], F32, tag="rz", name="rz")
                zbase = avz[:]
                pstep = zbase.ap[0][0]
                zin = bass.AP(zbase.tensor, zbase.offset + 64,
                              [[pstep, 128], [65, 2]])
                nc.vector.reciprocal(rz2[:], zin)
                # one multiply for both heads: avz cols {0:64, 65:129}
                # times rz2 broadcast 64-wide along the head axis
                avin = bass.AP(zbase.tensor, zbase.offset,
                               [[pstep, 128], [65, 2], [1, 64]])
                rzb = rz2[:]
                rzin = bass.AP(rzb.tensor, rzb.offset,
                               [[rzb.ap[0][0], 128], [1, 2], [0, 64]])
                nc.vector.tensor_mul(aT_sb[t][:, hA * 64:hA * 64 + 128],
                                     avin, rzin)

            for g, t in steps:
                sA = s_ps.tile([128, 256], F32, tag="s", name="s")
                sB = s_ps.tile([128, 256], F32, tag="s", name="s")
                for kb in range(2):
                    kc = (t + kb) * 128
                    nc.tensor.matmul(sA[:, kb * 128:(kb + 1) * 128],
                                     k_sb[g][0:64, kc:kc + 128],
                                     q_sb[g][0:64, t * 128:(t + 1) * 128],
                                     start=True, stop=True)
                    nc.tensor.matmul(sB[:, kb * 128:(kb + 1) * 128],
                                     k_sb[g][64:128, kc:kc + 128],
                                     q_sb[g][64:128, t * 128:(t + 1) * 128],
                                     start=True, stop=True)
                eA = soft.tile([128, 256], BF16, tag="eA", name="eA")
                eB = soft.tile([128, 256], BF16, tag="eB", name="eB")
                nc.scalar.activation(eA[:], sA[:],
                                     mybir.ActivationFunctionType.Exp)
                nc.scalar.activation(eB[:], sB[:],
                                     mybir.ActivationFunctionType.Exp)
                pA = soft.tile([128, 256], BF16, tag="pA", name="pA")
                pB = soft.tile([128, 256], BF16, tag="pB", name="pB")
                msk = mask_sb[:, t * 256:(t + 1) * 256]
                nc.gpsimd.tensor_mul(pA[:], eA[:], msk)
                nc.vector.tensor_mul(pB[:], eB[:], msk)
                avz = av_ps.tile([128, 130], F32, tag="av", name="av")
                if pend is not None:
                    emit_av(pend)
                pend = (g, t, pA, pB, avz)
            emit_av(pend)

            # ---- transpose a^T back to d-major ----------------------
            for t in range(2):
                for g in range(4):
                    tp = av_ps.tile([128, 128], BF16, tag="av", name="tp")
                    nc.tensor.transpose(tp[:],
                                        aT_sb[t][:, g * 128:(g + 1) * 128],
                                        ident[:])
                    nc.vector.tensor_copy(a_sb[g][:, t * 128:(t + 1) * 128],
                                          tp[:])

            # ---- output projection (tail, reuses pp banks) ----------
            for gg in range(4):
                ps = pp.tile([128, 512], F32, tag="pp", name="pp")
                for g in range(4):
                    nc.tensor.matmul(ps[:, :SEQ_PER_CORE],
                                     wo_sb[:, g * 512 + gg * 128:
                                           g * 512 + gg * 128 + 128],
                                     a_sb[g][:],
                                     start=(g == 0), stop=(g == 3))
                dst = o_all[:, gg * 256:(gg + 1) * 256]
                if gg % 2 == 0:
                    nc.scalar.activation(dst, ps[:, :SEQ_PER_CORE],
                                         mybir.ActivationFunctionType.Identity,
                                         bias=bias_sb[:, 4 + gg:5 + gg])
                else:
                    nc.vector.tensor_scalar_add(dst, ps[:, :SEQ_PER_CORE],
                                                bias_sb[:, 4 + gg:5 + gg])
                    # ship each completed half while the next one computes
                    nc.sync.dma_start(outT[:, (gg - 1) * 256:(gg + 1) * 256],
                                      o_all[:, (gg - 1) * 256:(gg + 1) * 256])

    nc.compile()
    return nc


def _band_mask(T):
    """Binary S^T masks [128 keys, 256 (2 kb blocks of 128)] for global
    query tile T (0..15)."""
    j = np.arange(128)[:, None]
    r = np.arange(128)[None, :]
    out = np.zeros((128, 256), np.float32)
    for kb in range(2):
        band = (j >= r) if kb == 0 else (j <= r)
        jg = T * 128 - W + kb * 128 + j
        valid = band & (jg >= 0) & (jg < S)
        out[:, kb * 128:(kb + 1) * 128] = valid
    return out


def _prep_inputs(x, Wq, bq, Wk, bk, Wv, bv, Wo, bo):
    bf = ml_dtypes.bfloat16
    f32 = np.float32

    def pack_w(Wm, scale=1.0):
        wT = np.asarray(Wm, f32).T * scale          # [512 in, 512 out]
        return np.ascontiguousarray(
            wT.reshape(4, 128, 512).transpose(1, 0, 2).reshape(128, 2048)
            .astype(bf))

    # wk packed g-major (out-group g at cols g*512, k-chunk kk at +kk*128)
    # so per-group DMA chunks arrive in consumption order
    wkT = np.asarray(Wk, f32).T
    wk4 = np.ascontiguousarray(
        wkT.reshape(4, 128, 4, 128).transpose(1, 2, 0, 3).reshape(128, 2048)
        .astype(bf))
    wq4 = pack_w(Wq, SCALE)
    wv4 = pack_w(Wv)
    wo4 = pack_w(Wo)
    bo_eff = np.asarray(bo, f32) + np.asarray(Wo, f32) @ np.asarray(bv, f32)
    bias = np.zeros((128, 12), f32)
    bias[:, 0:4] = np.asarray(bk, f32).reshape(4, 128).T
    bias[:, 4:8] = bo_eff.reshape(4, 128).T
    bias[:, 8:12] = (np.asarray(bq, f32) * SCALE).reshape(4, 128).T

    identw = np.eye(128, dtype=f32).astype(bf)
    xf = np.asarray(x, f32)
    in_maps = []
    for c in range(N_CORES):
        klo = c * SEQ_PER_CORE - W
        lo, hi = max(0, klo), min(S, klo + CHUNK)
        xT_c = np.zeros((D, CHUNK), f32)
        xT_c[:, lo - klo:hi - klo] = xf[0, lo:hi, :].T
        x4 = np.ascontiguousarray(
            xT_c.reshape(4, 128, CHUNK).transpose(1, 0, 2)
            .reshape(128, 4 * CHUNK).astype(bf))
        m = np.concatenate([_band_mask(c * 2), _band_mask(c * 2 + 1)],
                           axis=1).astype(bf)
        in_maps.append({
            "x4": x4, "mask4": np.ascontiguousarray(m), "bias": bias,
            "wq4": wq4, "wk4": wk4, "wv4": wv4, "wo4": wo4,
            "identw": identw,
        })
    return in_maps


def kernel(x, Wq, bq, Wk, bk, Wv, bv, Wo, bo):
    if "nc" not in _CACHE:
        _CACHE["nc"] = _build_program()
    nc = _CACHE["nc"]
    in_maps = _prep_inputs(x, Wq, bq, Wk, bk, Wv, bv, Wo, bo)
    res = bass_utils.run_bass_kernel_spmd(nc, in_maps,
                                          core_ids=list(range(N_CORES)))
    out = np.empty((1, S, D), np.float32)
    for c in range(N_CORES):
        arr = np.asarray(res.results[c]["outT"]).astype(np.float32)
        chunk = arr.reshape(128, 4, SEQ_PER_CORE).transpose(1, 0, 2) \
                   .reshape(D, SEQ_PER_CORE).T
        out[0, c * SEQ_PER_CORE:(c + 1) * SEQ_PER_CORE, :] = chunk
    return out
